# revision 44
# baseline (speedup 1.0000x reference)
"""GQA kernel for Trainium2, 8 NeuronCores (v3).

Problem: B=4, S=1024, D=2048, 32 q-heads, 8 kv-heads, head_dim=64, fp32 io.

Sharding: TP-2 over heads x DP-4 over batch. Core c handles batch c//2 and
(for tp = c%2) q-heads [16*tp, 16*tp+16) / kv-heads [4*tp, 4*tp+4). Each core
produces a partial output [1024, 2048] (its heads' contribution to ctx @ Wo);
host sums the two partials per batch and adds bo.

Key structure (all matmul operands bf16, psum fp32):
 - x arrives pre-transposed from the host; all weights arrive in exact SBUF
   layout inside one bf16 blob, split into DMAs ordered so the PE starts
   ~4us in and is never starved.
 - Phase A: K proj interleaved with V (token tiles 0-3, lagged 2 chunks so
   wv's DMA can land), then V tiles 4-7, then Q pairs 0-1.
 - Phase B th=0: per pair p: scores -> exp(ACT) -> PV lagged 2 blocks; the
   Q projection of pair p+2 is interleaved 2 chunks per block as PE filler
   (the exp stream on ACT is the throughput limit; filler keeps PE ahead).
 - Phase B th=1: same, with O-projection of token tiles 0-3 as filler.
 - Phase C: O-projection of token tiles 4-7; psum reuses the same pools as
   B (no pool transition barrier). One [128,2048] fp32 store per tile,
   except the last tile stores per-512 slice to shorten the drain tail.
"""

import time

import numpy as np
import ml_dtypes

import concourse.bass as bass
import concourse.mybir as mybir
from concourse import bacc
from concourse.tile import TileContext
from concourse.bass_utils import run_bass_kernel_spmd

F32 = mybir.dt.float32
FP8 = mybir.dt.float8e4
DRM = mybir.MatmulPerfMode.DoubleRow
BF16 = mybir.dt.bfloat16

S = 1024          # sequence length
D = 2048          # d_model
NH = 16           # q heads per core
NKV = 4           # kv heads per core
HD = 64           # head dim
QF = NH * HD      # 1024 q features per core
KF = NKV * HD     # 256 kv features per core
KC = D // 128     # 16 contraction chunks of d_model
TT = S // 128     # 8 token tiles
TH = S // 512     # 2 token halves
SCALE8 = 1.0 / 8.0 / 64.0  # 1/sqrt(64) / WSCALE (q only)

# pair p -> (lo head, hi head) local q-head indices; lo heads have kv parity 0,
# hi heads kv parity 1 (kv = h // 4; kv 0,2 -> rows 0:64 of kT group kv//2).
LO = [0, 1, 2, 3, 8, 9, 10, 11]
HI = [4, 5, 6, 7, 12, 13, 14, 15]
HEAD_PERM = []
for _p in range(8):
    HEAD_PERM.extend([LO[_p], HI[_p]])

# blob layout: per-partition element offsets (bf16). Order = DMA issue order.
# wk is split c0-5 / c6-10 / c11-15 and x into 6 pieces so the K/V chunk
# loop can chase the DMA stream without stalling.
_pieces = [
    ("wkA", KF),            # wk chunk 0
    ("xA", S),              # x chunk 0
    ("wkA1", 3 * KF),       # wk chunks 1-3
    ("xB1", S),             # x chunk 1
    ("xB2", 2 * S),         # x chunks 2-3
    ("wkB", 6 * KF),        # wk chunks 4-9
    ("xC1", 2 * S),         # x chunks 4-5
    ("xC2", 2 * S),         # x chunks 6-7
    ("wv", KC * KF),
    ("wkC", 6 * KF),        # wk chunks 10-15
    ("xD", 4 * S),          # x chunks 8-11
    ("xE", 4 * S),          # x chunks 12-15
    ("wo", 8 * D),
    ("iden", 128),
]
OFF = {}
_o = 0
for _n, _sz in _pieces:
    OFF[_n] = _o
    _o += _sz
BLOB_N = _o

# fp8 blob: DoubleRow layouts for the Q projection (weights prescaled x64;
# the x64^2 on q.k scores is folded into the exp scale).
DC = D // 256
_pieces8 = [
    ("x8a", 4 * 2048),      # x8 dc 0-3: [dc][i][n]
    ("wq8A", 2 * 2048),     # wq pairs 0-1: [pr][dc][i][m]
    ("x8b", 4 * 2048),      # x8 dc 4-7
    ("wq8B", 6 * 2048),     # wq pairs 2-7
]
OFF8 = {}
_o = 0
for _n, _sz in _pieces8:
    OFF8[_n] = _o
    _o += _sz
BLOB8_N = _o
WSCALE = 64.0

# bias tensor layout (fp32, [128, 270])
BOFF_Q = 0      # 8 cols, col p = pair p
BOFF_K = 8      # 2 cols, col g
BOFF_V = 10     # 256 cols, bv broadcast across partitions
BOFF_ONE = 266  # 4 cols of ones
BIAS_N = 270

_CACHE = {}
LAST_RUN_NS = None


def _build():
    if "nc" in _CACHE:
        return _CACHE["nc"]

    nc = bacc.Bacc("TRN2", target_bir_lowering=False, debug=False)

    blob = nc.dram_tensor("blob", [128, BLOB_N], BF16, kind="ExternalInput").ap()
    blob8 = nc.dram_tensor("blob8", [128, BLOB8_N], FP8, kind="ExternalInput").ap()
    biasd = nc.dram_tensor("biasd", [128, BIAS_N], F32, kind="ExternalInput").ap()
    out = nc.dram_tensor("out", [S, D], F32, kind="ExternalOutput").ap()

    with TileContext(nc) as tc:
        with tc.tile_pool(name="main", bufs=1) as mp:
            wk_sb = mp.tile([128, KC * KF], BF16, tag="wk")
            wv_sb = mp.tile([128, KC * KF], BF16, tag="wv")
            x_sb = mp.tile([128, KC * S], BF16, tag="x")
            wq8_sb = mp.tile([128, 8 * 2048], FP8, tag="wq8")
            x8_sb = mp.tile([128, DC * 2048], FP8, tag="x8")
            wo_sb = mp.tile([128, 8 * D], BF16, tag="wo")
            bias_sb = mp.tile([128, BIAS_N], F32, tag="bias")

            kT = [mp.tile([128, S], BF16, tag=f"kT{g}", name=f"kT{g}")
                  for g in range(2)]
            vaug = mp.tile([128, TT * 65 * NKV], BF16, tag="vaug")
            qT = [mp.tile([128, S], BF16, tag=f"qT{p}", name=f"qT{p}")
                  for p in range(8)]
            ctxT = [mp.tile([128, S], BF16, tag=f"ctxT{p}", name=f"ctxT{p}")
                    for p in range(8)]

            # ---- input DMAs, in arrival order ----
            def dma_in(dst, name, sz):
                nc.sync.dma_start(out=dst, in_=blob[:, OFF[name]:OFF[name] + sz])

            dma_in(wk_sb[:, 0:KF], "wkA", KF)
            dma_in(x_sb[:, 0:S], "xA", S)
            dma_in(wk_sb[:, KF:4 * KF], "wkA1", 3 * KF)
            dma_in(x_sb[:, S:2 * S], "xB1", S)
            dma_in(x_sb[:, 2 * S:4 * S], "xB2", 2 * S)
            dma_in(wk_sb[:, 4 * KF:10 * KF], "wkB", 6 * KF)
            dma_in(x_sb[:, 4 * S:6 * S], "xC1", 2 * S)
            dma_in(x_sb[:, 6 * S:8 * S], "xC2", 2 * S)
            dma_in(wv_sb[:], "wv", KC * KF)
            dma_in(wk_sb[:, 10 * KF:16 * KF], "wkC", 6 * KF)
            dma_in(x_sb[:, 8 * S:12 * S], "xD", 4 * S)

            def dma8(dst, name, sz):
                nc.sync.dma_start(out=dst, in_=blob8[:, OFF8[name]:OFF8[name] + sz])

            dma8(x8_sb[:, 0:4 * 2048], "x8a", 4 * 2048)
            dma8(wq8_sb[:, 0:2 * 2048], "wq8A", 2 * 2048)
            dma_in(x_sb[:, 12 * S:16 * S], "xE", 4 * S)
            dma8(x8_sb[:, 4 * 2048:8 * 2048], "x8b", 4 * 2048)
            nc.sync.dma_start(out=bias_sb[:], in_=biasd[:, :])
            dma8(wq8_sb[:, 2 * 2048:8 * 2048], "wq8B", 6 * 2048)
            dma_in(wo_sb[:], "wo", 8 * D)
            tid = mp.tile([128, 128], BF16, tag="tid")
            dma_in(tid[:], "iden", 128)

            def two(ap):
                return ap.rearrange("p (two m) -> p two m", two=2)

            def x8dc(dc):
                return two(x8_sb[:, 2048 * dc:2048 * (dc + 1)])

            def xsl(c, a, b):
                return x_sb[:, S * c + a:S * c + b]

            va = vaug[:].rearrange("p (t j f) -> p t j f", t=TT, j=NKV, f=65)
            bv_j = bias_sb[:, BOFF_V:BOFF_V + KF].rearrange(
                "p (j f) -> p j f", j=NKV)
            ones_j = bias_sb[:, BOFF_ONE:BOFF_ONE + NKV].rearrange(
                "p (j f) -> p j f", j=NKV)

            def k_chunk(c, pk):
                for g in range(2):
                    for th in range(TH):
                        nc.tensor.matmul(
                            pk[(g, th)][:],
                            wk_sb[:, KF * c + 128 * g:KF * c + 128 * (g + 1)],
                            xsl(c, 512 * th, 512 * (th + 1)),
                            start=(c == 0), stop=(c == KC - 1),
                        )

            def v_chunk(c, t4, pv4):
                # one chunk of V proj for 4 token tiles t4*4..t4*4+3
                for i in range(4):
                    t = 4 * t4 + i
                    nc.tensor.matmul(
                        pv4[i][:, 0:KF],
                        xsl(c, 128 * t, 128 * (t + 1)),
                        wv_sb[:, KF * c:KF * (c + 1)],
                        start=(c == 0), stop=(c == KC - 1),
                    )

            # ones columns depend only on the bias DMA: write them all
            # up-front so v_writeout is just the 4 adds (shorter WAR chain
            # when the V psum banks are recycled).
            for _t in range(TT):
                nc.vector.tensor_copy(va[:, _t, :, 64:65], ones_j)

            def v_writeout(t4, pv4):
                for i in range(4):
                    t = 4 * t4 + i
                    pv_j = pv4[i][:, 0:KF].rearrange("p (j f) -> p j f", j=NKV)
                    nc.vector.tensor_add(va[:, t, :, 0:64], pv_j, bv_j)

            # ---- Phase A1: K proj + V(t0-3) lagged 2 chunks, then V(t4-7)
            with (
                tc.tile_pool(name="psK", bufs=1, space="PSUM") as psK,
                tc.tile_pool(name="psV", bufs=1, space="PSUM") as psV,
            ):
                pk = {}
                for g in range(2):
                    for th in range(TH):
                        pk[(g, th)] = psK.tile(
                            [128, 512], F32, tag=f"pk{g}{th}", name=f"pk{g}{th}")
                # one full psum BANK per V accumulation group: a matmul with
                # start=True marks its whole 2KB zero-region pending-zero, so
                # two interleaved accumulation groups must never share a bank.
                pv03 = [psV.tile([128, 512], F32, tag=f"pva{i}", name=f"pva{i}")
                        for i in range(4)]
                for c in range(5):
                    k_chunk(c, pk)
                for c in range(5):
                    v_chunk(c, 0, pv03)
                for c in range(5, KC):
                    k_chunk(c, pk)
                    v_chunk(c, 0, pv03)
                v_writeout(0, pv03)
                for g in range(2):
                    for th in range(TH):
                        nc.scalar.activation(
                            kT[g][:, 512 * th:512 * (th + 1)], pk[(g, th)][:],
                            mybir.ActivationFunctionType.Identity,
                            bias=bias_sb[:, BOFF_K + g:BOFF_K + g + 1],
                            scale=1.0,
                        )
                def v_half(c, tb, pv2):
                    for i in range(2):
                        t = tb + i
                        nc.tensor.matmul(
                            pv2[i][:, 0:KF],
                            xsl(c, 128 * t, 128 * (t + 1)),
                            wv_sb[:, KF * c:KF * (c + 1)],
                            start=(c == 0), stop=(c == KC - 1),
                        )

                def v_half_out(tb, pv2):
                    for i in range(2):
                        t = tb + i
                        pv_j = pv2[i][:, 0:KF].rearrange(
                            "p (j f) -> p j f", j=NKV)
                        nc.vector.tensor_add(va[:, t, :, 0:64], pv_j, bv_j)

                # V tiles 4-7 in two 2-tile passes: the first reuses only
                # banks pva0/pva1, so it waits just the first two V03 adds.
                pv45 = [psV.tile([128, 512], F32, tag=f"pva{i}", name=f"pvb{i}")
                        for i in range(2)]
                for c in range(KC):
                    v_half(c, 4, pv45)
                v_half_out(4, pv45)
                pv67 = [psV.tile([128, 512], F32, tag=f"pva{i + 2}",
                                 name=f"pvc{i}")
                        for i in range(2)]
                for c in range(KC):
                    v_half(c, 6, pv67)
                v_half_out(6, pv67)

            # ---- Phase B (+ A2 Q pairs 0/1 + phase C t4-7, sharing pools)
            with (
                tc.tile_pool(name="epool", bufs=16) as ep,
                tc.tile_pool(name="npool", bufs=2) as npool,
                tc.tile_pool(name="osb", bufs=2) as osbp,
                tc.tile_pool(name="ps_sc", bufs=2, space="PSUM") as ps_sc,
                tc.tile_pool(name="ps_tr", bufs=2, space="PSUM") as ps_tr,
                tc.tile_pool(name="ps_fq", bufs=1, space="PSUM") as ps_fq,
                tc.tile_pool(name="ps_fo", bufs=1, space="PSUM") as ps_fo,
            ):
                fill_alt = [0]

                def fill_bank(name):
                    # Alternate the two 1-bank filler psums so consecutive
                    # accumulation groups double-buffer instead of
                    # serializing on the previous group's psum readout.
                    pool, tg = [(ps_fq, "fq"), (ps_fo, "fo")][fill_alt[0] % 2]
                    fill_alt[0] += 1
                    return pool.tile([128, 512], F32, tag=tg, name=name)

                def q_half_item(p, thh, dc, box):
                    """One DoubleRow d-chunk of the Q projection of
                    (pair p, half thh); bias-add to qT on the last chunk."""
                    if dc == 0:
                        box["pq"] = fill_bank(f"fq{p}_{thh}")
                    pq = box["pq"]
                    nc.tensor.matmul(
                        pq[:],
                        two(wq8_sb[:, 2048 * p + 256 * dc:
                                   2048 * p + 256 * (dc + 1)]),
                        x8dc(dc)[:, :, 512 * thh:512 * (thh + 1)],
                        start=(dc == 0), stop=(dc == DC - 1),
                        perf_mode=DRM,
                    )
                    if dc == DC - 1:
                        nc.vector.tensor_scalar_add(
                            qT[p][:, 512 * thh:512 * (thh + 1)], pq[:],
                            bias_sb[:, BOFF_Q + p:BOFF_Q + p + 1],
                        )

                o_tiles = {}

                def o_item(t, nf, k, box):
                    """One pair's contribution to out tile t, nf slot nf;
                    copy to o_sb on the last pair, store on the last nf.
                    Alternates the fo/fq psum banks so consecutive singles
                    double-buffer instead of serializing on the copy-out."""
                    if k == 0:
                        box["po"] = fill_bank(f"fo{t}_{nf}")
                    po = box["po"]
                    nc.tensor.matmul(
                        po[:],
                        ctxT[k][:, 128 * t:128 * (t + 1)],
                        wo_sb[:, 2048 * k + 512 * nf:2048 * k + 512 * (nf + 1)],
                        start=(k == 0), stop=(k == 7),
                    )
                    if k == 7:
                        if nf == 0:
                            o_tiles[t] = osbp.tile([128, D], F32, tag="osb",
                                                   name=f"osb{t}")
                        nc.vector.tensor_copy(
                            o_tiles[t][:, 512 * nf:512 * (nf + 1)], po[:])
                        if nf == 3:
                            nc.sync.dma_start(
                                out=out[128 * t:128 * (t + 1), :],
                                in_=o_tiles[t][:])

                def make_queue_filler(queue, slot_total):
                    # 18 calls per unit (2 in the tail between the last PVs);
                    # front-load zeros so the tail always has filler to hide
                    # the last exps' latency.
                    base = slot_total // 18
                    extra = slot_total - 18 * base
                    counts = [base + (1 if i >= 18 - extra else 0)
                              for i in range(18)]

                    def fill(i):
                        for _ in range(counts[i]):
                            if queue:
                                queue.pop(0)()
                    return fill

                # A2: Q pairs 0 and 1, th=0 halves, in one psc-tag tile
                # (keeps ps_fq free of WARs for the th0 sweep's first filler).
                pqA = ps_sc.tile([128, 1024], F32, tag="psc", name="pqA")
                for p in range(2):
                    for dc in range(DC):
                        nc.tensor.matmul(
                            pqA[:, 512 * p:512 * (p + 1)],
                            two(wq8_sb[:, 2048 * p + 256 * dc:
                                       2048 * p + 256 * (dc + 1)]),
                            x8dc(dc)[:, :, 0:512],
                            start=(dc == 0), stop=(dc == DC - 1),
                            perf_mode=DRM,
                        )
                    nc.scalar.activation(
                        qT[p][:, 0:512], pqA[:, 512 * p:512 * (p + 1)],
                        mybir.ActivationFunctionType.Identity,
                        bias=bias_sb[:, BOFF_Q + p:BOFF_Q + p + 1],
                        scale=1.0,
                    )

                def pv_group(p, th, g, es, kvlo, kvhi):
                    """Group g (0..7): head-half hh = g//4 (lo/hi), query
                    chunk sqc = g%4. Accumulates e.T @ vaug over the 8 key
                    blocks into a [128, 65] psum (col 64 = denominator),
                    then per-partition reciprocal+scale on DVE and a PE
                    transpose back to feature-major ctxT."""
                    hh, sqc = g // 4, g % 4
                    kv = kvlo if hh == 0 else kvhi
                    pvf = fill_bank(f"pv{p}_{th}_{g}")
                    for blk in range(TT):
                        nc.tensor.matmul(
                            pvf[:, 0:65],
                            es[blk][:, 512 * hh + 128 * sqc:
                                    512 * hh + 128 * (sqc + 1)],
                            vaug[:, 260 * blk + 65 * kv:260 * blk + 65 * kv + 65],
                            start=(blk == 0), stop=(blk == TT - 1),
                        )
                    rec = npool.tile([128, 1], F32, tag="rec", name=f"r{p}{th}{g}")
                    nc.vector.reciprocal(rec[:], pvf[:, 64:65])
                    ctx_sb = npool.tile([128, 64], BF16, tag="cs",
                                        name=f"cs{p}{th}{g}")
                    nc.vector.tensor_scalar_mul(ctx_sb[:], pvf[:, 0:64], rec[:])
                    ptr = ps_tr.tile([64, 128], BF16, tag="ptr",
                                     name=f"ptr{p}{th}{g}")
                    nc.tensor.transpose(ptr[:], ctx_sb[:], tid[:])
                    nc.vector.tensor_copy(
                        ctxT[p][64 * hh:64 * (hh + 1),
                                512 * th + 128 * sqc:512 * th + 128 * (sqc + 1)],
                        ptr[:])

                def unit(p, th, filler):
                    """Attention for (pair p, query half th), PV flipped.
                    `filler(i)`, i in 0..17, emits interleaved PE filler."""
                    glo, ghi = LO[p] // 4 // 2, HI[p] // 4 // 2
                    kvlo, kvhi = LO[p] // 4, HI[p] // 4
                    es = [None] * TT

                    for blk in range(TT):
                        psc = ps_sc.tile([128, 1024], F32, tag="psc",
                                         name=f"psc{p}_{th}_{blk}")
                        nc.tensor.matmul(
                            psc[:, 0:512],
                            kT[glo][0:64, 128 * blk:128 * (blk + 1)],
                            qT[p][0:64, 512 * th:512 * (th + 1)],
                            start=True, stop=True,
                        )
                        nc.tensor.matmul(
                            psc[:, 512:1024],
                            kT[ghi][64:128, 128 * blk:128 * (blk + 1)],
                            qT[p][64:128, 512 * th:512 * (th + 1)],
                            start=True, stop=True,
                        )
                        e = ep.tile([128, 1024], BF16, tag="e",
                                    name=f"e{p}_{th}_{blk}")
                        nc.scalar.activation(
                            e[:], psc[:], mybir.ActivationFunctionType.Exp,
                            bias=0.0, scale=SCALE8,
                        )
                        es[blk] = e
                        filler(2 * blk)
                        filler(2 * blk + 1)
                    # all PV groups read every e tile, so they start after
                    # exp(7); the two tail filler slots bridge that latency.
                    filler(16)
                    filler(17)
                    for g in range(8):
                        pv_group(p, th, g, es, kvlo, kvhi)

                # --- th=0 sweep. Filler queue: Q th0-halves of pairs 2-7,
                # then Q th1-halves of pairs 0-1 (needed before th1 sweep).
                q0 = []
                for fp in range(2, 8):
                    box = {}
                    for dc in range(DC):
                        q0.append(lambda fp=fp, dc=dc, box=box:
                                  q_half_item(fp, 0, dc, box))
                for fp in list(range(2)) + list(range(2, 8)):
                    box = {}
                    for dc in range(DC):
                        q0.append(lambda fp=fp, dc=dc, box=box:
                                  q_half_item(fp, 1, dc, box))
                th0_counts = [20, 20, 14, 14, 14, 14, 11, 5]
                for p in range(8):
                    unit(p, 0, make_queue_filler(q0, th0_counts[p]))
                assert not q0

                # --- th=1 sweep. Filler queue: O projection of tiles 0-3.
                q1 = []
                for t in range(4):
                    for nf in range(4):
                        box = {}
                        for k in range(8):
                            q1.append(lambda t=t, nf=nf, k=k, box=box:
                                      o_item(t, nf, k, box))
                for p in [7] + list(range(7)):
                    unit(p, 1, make_queue_filler(q1, 16))
                assert not q1

                # --- Phase C: O proj of token tiles 4-7. The last tile
                # streams per-512-column: matmuls -> copy -> store per nf, so
                # the final store drains right behind the final matmul.
                for t in range(4, TT):
                    for half in range(2):
                        pt = ps_sc.tile([128, 1024], F32, tag="psc",
                                        name=f"poC{t}_{half}")
                        if half == 0:
                            o_tiles[t] = osbp.tile([128, D], F32, tag="osb",
                                                   name=f"osbC{t}")
                        for j in range(2):
                            nf = 2 * half + j
                            for k in range(8):
                                nc.tensor.matmul(
                                    pt[:, 512 * j:512 * (j + 1)],
                                    ctxT[k][:, 128 * t:128 * (t + 1)],
                                    wo_sb[:, 2048 * k + 512 * nf:
                                          2048 * k + 512 * (nf + 1)],
                                    start=(k == 0), stop=(k == 7),
                                )
                            sl = o_tiles[t][:, 512 * nf:512 * (nf + 1)]
                            if nf % 2 == 0:
                                nc.scalar.copy(sl, pt[:, 512 * j:512 * (j + 1)])
                            else:
                                nc.vector.tensor_copy(
                                    sl, pt[:, 512 * j:512 * (j + 1)])
                            # per-512-column stores: each slice drains right
                            # behind its copy, so the final store isn't
                            # queued behind a whole-tile transfer.
                            nc.sync.dma_start(
                                out=out[128 * t:128 * (t + 1),
                                        512 * nf:512 * (nf + 1)],
                                in_=sl)

    nc.compile()
    _CACHE["nc"] = nc
    return nc


def _prep_core_inputs(c, x, Wq, bq, Wk, bk, Wv, bv, Wo, bo):
    tp = c % 2
    b = c // 2
    hperm = [16 * tp + h for h in HEAD_PERM]

    # xT chunks: region[dp, 1024*c + t] = x[b][t, 128*c + dp]
    xr = np.ascontiguousarray(x[b].T).reshape(KC, 128, S).transpose(1, 0, 2)
    xr = xr.reshape(128, KC * S)

    wk_r = Wk[:, KF * tp:KF * (tp + 1)].reshape(KC, 128, KF)
    wk_r = wk_r.transpose(1, 0, 2).reshape(128, KC * KF)
    wv_r = Wv[:, KF * tp:KF * (tp + 1)].reshape(KC, 128, KF)
    wv_r = wv_r.transpose(1, 0, 2).reshape(128, KC * KF)

    f8 = mybir.dt.np(FP8)
    xT = np.ascontiguousarray(x[b].T)
    # x8[k, 2048*dc + 1024*i + n] = xT[256*dc + 128*i + k, n]
    x8 = xT.reshape(DC, 2, 128, S).transpose(2, 0, 1, 3).reshape(128, DC * 2048)
    wq_perm = WSCALE * Wq.reshape(D, 32, HD)[:, hperm, :].reshape(D, QF)
    # wq8[k, 2048*pr + 256*dc + 128*i + m] = 64*Wq_perm[256dc+128i+k, 128pr+m]
    wq8 = wq_perm.reshape(DC, 2, 128, 8, 128).transpose(2, 3, 0, 1, 4)
    wq8 = wq8.reshape(128, 8 * 2048)
    blob8 = np.concatenate(
        [x8[:, 0:4 * 2048], wq8[:, 0:2 * 2048],
         x8[:, 4 * 2048:8 * 2048], wq8[:, 2 * 2048:8 * 2048]],
        axis=1).astype(f8)

    wo_perm = Wo.reshape(32, HD, D)[hperm].reshape(QF, D)
    wo_r = wo_perm.reshape(8, 128, D).transpose(1, 0, 2).reshape(128, 8 * D)

    blob = np.concatenate(
        [wk_r[:, 0:KF],              # wkA
         xr[:, 0:S],                 # xA
         wk_r[:, KF:4 * KF],         # wkA1
         xr[:, S:2 * S],             # xB1
         xr[:, 2 * S:4 * S],         # xB2
         wk_r[:, 4 * KF:10 * KF],    # wkB
         xr[:, 4 * S:6 * S],         # xC1
         xr[:, 6 * S:8 * S],         # xC2
         wv_r,                       # wv
         wk_r[:, 10 * KF:16 * KF],   # wkC
         xr[:, 8 * S:12 * S],        # xD
         xr[:, 12 * S:16 * S],       # xE
         wo_r,
         np.eye(128, dtype=np.float32)],
        axis=1).astype(ml_dtypes.bfloat16)

    bq_c = WSCALE * bq.reshape(32, HD)[hperm].reshape(8, 128).T   # [128, 8]
    bk_c = bk[KF * tp:KF * (tp + 1)].reshape(2, 128).T            # [128, 2]
    bv_c = np.tile(bv[KF * tp:KF * (tp + 1)][None, :], (128, 1))  # [128, 256]
    ones = np.ones((128, 4), np.float32)
    biasd = np.concatenate([bq_c, bk_c, bv_c, ones], axis=1).astype(np.float32)

    return {
        "blob": np.ascontiguousarray(blob),
        "blob8": np.ascontiguousarray(blob8),
        "biasd": np.ascontiguousarray(biasd),
    }


def kernel(x, Wq, bq, Wk, bk, Wv, bv, Wo, bo):
    global LAST_RUN_NS
    nc = _build()
    in_maps = [
        _prep_core_inputs(c, x, Wq, bq, Wk, bk, Wv, bv, Wo, bo) for c in range(8)
    ]
    t0 = time.perf_counter_ns()
    res = run_bass_kernel_spmd(nc, in_maps, list(range(8)))
    LAST_RUN_NS = time.perf_counter_ns() - t0
    parts = [res.results[c]["out"] for c in range(8)]
    out = np.empty((4, S, D), np.float32)
    for b in range(4):
        out[b] = parts[2 * b] + parts[2 * b + 1] + bo[None, :]
    return out


# revision 51
# speedup vs baseline: 1.0226x; 1.0226x over previous
"""GQA kernel for Trainium2, 8 NeuronCores (v3).

Problem: B=4, S=1024, D=2048, 32 q-heads, 8 kv-heads, head_dim=64, fp32 io.

Sharding: TP-2 over heads x DP-4 over batch. Core c handles batch c//2 and
(for tp = c%2) q-heads [16*tp, 16*tp+16) / kv-heads [4*tp, 4*tp+4). Each core
produces a partial output [1024, 2048] (its heads' contribution to ctx @ Wo);
host sums the two partials per batch and adds bo.

Key structure (all matmul operands bf16, psum fp32):
 - x arrives pre-transposed from the host; all weights arrive in exact SBUF
   layout inside one bf16 blob, split into DMAs ordered so the PE starts
   ~4us in and is never starved.
 - Phase A: K proj interleaved with V (token tiles 0-3, lagged 2 chunks so
   wv's DMA can land), then V tiles 4-7, then Q pairs 0-1.
 - Phase B th=0: per pair p: scores -> exp(ACT) -> PV lagged 2 blocks; the
   Q projection of pair p+2 is interleaved 2 chunks per block as PE filler
   (the exp stream on ACT is the throughput limit; filler keeps PE ahead).
 - Phase B th=1: same, with O-projection of token tiles 0-3 as filler.
 - Phase C: O-projection of token tiles 4-7; psum reuses the same pools as
   B (no pool transition barrier). One [128,2048] fp32 store per tile,
   except the last tile stores per-512 slice to shorten the drain tail.
"""

import time

import numpy as np
import ml_dtypes

import concourse.bass as bass
import concourse.mybir as mybir
from concourse import bacc
from concourse.tile import TileContext
from concourse.bass_utils import run_bass_kernel_spmd

F32 = mybir.dt.float32
FP8 = mybir.dt.float8e4
DRM = mybir.MatmulPerfMode.DoubleRow
BF16 = mybir.dt.bfloat16

S = 1024          # sequence length
D = 2048          # d_model
NH = 16           # q heads per core
NKV = 4           # kv heads per core
HD = 64           # head dim
QF = NH * HD      # 1024 q features per core
KF = NKV * HD     # 256 kv features per core
KC = D // 128     # 16 contraction chunks of d_model
TT = S // 128     # 8 token tiles
TH = S // 512     # 2 token halves
SCALE8 = 1.0 / 8.0 / 64.0  # 1/sqrt(64) / WSCALE (q only)

# pair p -> (lo head, hi head) local q-head indices; lo heads have kv parity 0,
# hi heads kv parity 1 (kv = h // 4; kv 0,2 -> rows 0:64 of kT group kv//2).
LO = [0, 1, 2, 3, 8, 9, 10, 11]
HI = [4, 5, 6, 7, 12, 13, 14, 15]
HEAD_PERM = []
for _p in range(8):
    HEAD_PERM.extend([LO[_p], HI[_p]])

# blob layout: per-partition element offsets (bf16). Order = DMA issue order.
# wk is split c0-5 / c6-10 / c11-15 and x into 6 pieces so the K/V chunk
# loop can chase the DMA stream without stalling.
_pieces = [
    ("wkA", KF),            # wk chunk 0
    ("xA", S),              # x chunk 0
    ("wkA1", 3 * KF),       # wk chunks 1-3
    ("xB1", S),             # x chunk 1
    ("xB2", 2 * S),         # x chunks 2-3
    ("wkB", 6 * KF),        # wk chunks 4-9
    ("xC1", 2 * S),         # x chunks 4-5
    ("xC2", 2 * S),         # x chunks 6-7
    ("wv", KC * KF),
    ("wkC", 6 * KF),        # wk chunks 10-15
    ("xD", 4 * S),          # x chunks 8-11
    ("xE", 4 * S),          # x chunks 12-15
    ("wo", 8 * D),
    ("iden", 128),
]
OFF = {}
_o = 0
for _n, _sz in _pieces:
    OFF[_n] = _o
    _o += _sz
BLOB_N = _o

# fp8 blob: DoubleRow layouts for the Q projection (weights prescaled x64;
# the x64^2 on q.k scores is folded into the exp scale).
DC = D // 256
_pieces8 = [
    ("x8a", 4 * 2048),      # x8 dc 0-3: [dc][i][n]
    ("wq8A", 2 * 2048),     # wq pairs 0-1: [pr][dc][i][m]
    ("x8b", 4 * 2048),      # x8 dc 4-7
    ("wq8B", 6 * 2048),     # wq pairs 2-7
]
OFF8 = {}
_o = 0
for _n, _sz in _pieces8:
    OFF8[_n] = _o
    _o += _sz
BLOB8_N = _o
WSCALE = 64.0

# bias tensor layout (fp32, [128, 270])
BOFF_Q = 0      # 8 cols, col p = pair p
BOFF_K = 8      # 2 cols, col g
BOFF_V = 10     # 256 cols, bv broadcast across partitions
BOFF_ONE = 266  # 4 cols of ones
BIAS_N = 270

_CACHE = {}
LAST_RUN_NS = None


def _build():
    if "nc" in _CACHE:
        return _CACHE["nc"]

    nc = bacc.Bacc("TRN2", target_bir_lowering=False, debug=False)

    blob = nc.dram_tensor("blob", [128, BLOB_N], BF16, kind="ExternalInput").ap()
    blob8 = nc.dram_tensor("blob8", [128, BLOB8_N], FP8, kind="ExternalInput").ap()
    biasd = nc.dram_tensor("biasd", [128, BIAS_N], F32, kind="ExternalInput").ap()
    out = nc.dram_tensor("out", [S, D], F32, kind="ExternalOutput").ap()

    with TileContext(nc) as tc:
        with tc.tile_pool(name="main", bufs=1) as mp:
            wk_sb = mp.tile([128, KC * KF], BF16, tag="wk")
            wv_sb = mp.tile([128, KC * KF], BF16, tag="wv")
            x_sb = mp.tile([128, KC * S], BF16, tag="x")
            wq8_sb = mp.tile([128, 8 * 2048], FP8, tag="wq8")
            x8_sb = mp.tile([128, DC * 2048], FP8, tag="x8")
            wo_sb = mp.tile([128, 8 * D], BF16, tag="wo")
            bias_sb = mp.tile([128, BIAS_N], F32, tag="bias")

            kT = [mp.tile([128, S], BF16, tag=f"kT{g}", name=f"kT{g}")
                  for g in range(2)]
            vaug = mp.tile([128, TT * 65 * NKV], BF16, tag="vaug")
            qT = [mp.tile([128, S], BF16, tag=f"qT{p}", name=f"qT{p}")
                  for p in range(8)]
            ctxT = [mp.tile([128, S], BF16, tag=f"ctxT{p}", name=f"ctxT{p}")
                    for p in range(8)]

            # ---- input DMAs, in arrival order ----
            def dma_in(dst, name, sz):
                nc.sync.dma_start(out=dst, in_=blob[:, OFF[name]:OFF[name] + sz])

            dma_in(wk_sb[:, 0:KF], "wkA", KF)
            dma_in(x_sb[:, 0:S], "xA", S)
            dma_in(wk_sb[:, KF:4 * KF], "wkA1", 3 * KF)
            dma_in(x_sb[:, S:2 * S], "xB1", S)
            dma_in(x_sb[:, 2 * S:4 * S], "xB2", 2 * S)
            dma_in(wk_sb[:, 4 * KF:10 * KF], "wkB", 6 * KF)
            dma_in(x_sb[:, 4 * S:6 * S], "xC1", 2 * S)
            dma_in(x_sb[:, 6 * S:8 * S], "xC2", 2 * S)
            dma_in(wv_sb[:], "wv", KC * KF)
            dma_in(wk_sb[:, 10 * KF:16 * KF], "wkC", 6 * KF)
            dma_in(x_sb[:, 8 * S:12 * S], "xD", 4 * S)

            def dma8(dst, name, sz):
                nc.sync.dma_start(out=dst, in_=blob8[:, OFF8[name]:OFF8[name] + sz])

            dma8(x8_sb[:, 0:4 * 2048], "x8a", 4 * 2048)
            dma8(wq8_sb[:, 0:2 * 2048], "wq8A", 2 * 2048)
            dma_in(x_sb[:, 12 * S:16 * S], "xE", 4 * S)
            dma8(x8_sb[:, 4 * 2048:8 * 2048], "x8b", 4 * 2048)
            nc.sync.dma_start(out=bias_sb[:], in_=biasd[:, :])
            dma8(wq8_sb[:, 2 * 2048:8 * 2048], "wq8B", 6 * 2048)
            dma_in(wo_sb[:], "wo", 8 * D)
            tid = mp.tile([128, 128], BF16, tag="tid")
            dma_in(tid[:], "iden", 128)

            def two(ap):
                return ap.rearrange("p (two m) -> p two m", two=2)

            def x8dc(dc):
                return two(x8_sb[:, 2048 * dc:2048 * (dc + 1)])

            def xsl(c, a, b):
                return x_sb[:, S * c + a:S * c + b]

            va = vaug[:].rearrange("p (t j f) -> p t j f", t=TT, j=NKV, f=65)
            bv_j = bias_sb[:, BOFF_V:BOFF_V + KF].rearrange(
                "p (j f) -> p j f", j=NKV)
            ones_j = bias_sb[:, BOFF_ONE:BOFF_ONE + NKV].rearrange(
                "p (j f) -> p j f", j=NKV)

            def k_chunk(c, pk):
                for g in range(2):
                    for th in range(TH):
                        nc.tensor.matmul(
                            pk[(g, th)][:],
                            wk_sb[:, KF * c + 128 * g:KF * c + 128 * (g + 1)],
                            xsl(c, 512 * th, 512 * (th + 1)),
                            start=(c == 0), stop=(c == KC - 1),
                        )

            def v_chunk(c, t4, pv4):
                # one chunk of V proj for 4 token tiles t4*4..t4*4+3
                for i in range(4):
                    t = 4 * t4 + i
                    nc.tensor.matmul(
                        pv4[i][:, 0:KF],
                        xsl(c, 128 * t, 128 * (t + 1)),
                        wv_sb[:, KF * c:KF * (c + 1)],
                        start=(c == 0), stop=(c == KC - 1),
                    )

            # ones columns depend only on the bias DMA: write them all
            # up-front so v_writeout is just the 4 adds (shorter WAR chain
            # when the V psum banks are recycled).
            for _t in range(TT):
                nc.vector.tensor_copy(va[:, _t, :, 64:65], ones_j)

            def v_writeout(t4, pv4):
                for i in range(4):
                    t = 4 * t4 + i
                    pv_j = pv4[i][:, 0:KF].rearrange("p (j f) -> p j f", j=NKV)
                    nc.vector.tensor_add(va[:, t, :, 0:64], pv_j, bv_j)

            # ---- Phase A1: K proj + V(t0-3) lagged 2 chunks, then V(t4-7)
            with (
                tc.tile_pool(name="psK", bufs=1, space="PSUM") as psK,
                tc.tile_pool(name="psV", bufs=1, space="PSUM") as psV,
            ):
                pk = {}
                for g in range(2):
                    for th in range(TH):
                        pk[(g, th)] = psK.tile(
                            [128, 512], F32, tag=f"pk{g}{th}", name=f"pk{g}{th}")
                # one full psum BANK per V accumulation group: a matmul with
                # start=True marks its whole 2KB zero-region pending-zero, so
                # two interleaved accumulation groups must never share a bank.
                pv03 = [psV.tile([128, 512], F32, tag=f"pva{i}", name=f"pva{i}")
                        for i in range(4)]
                for c in range(5):
                    k_chunk(c, pk)
                for c in range(5):
                    v_chunk(c, 0, pv03)
                for c in range(5, KC):
                    k_chunk(c, pk)
                    v_chunk(c, 0, pv03)
                v_writeout(0, pv03)
                for g in range(2):
                    for th in range(TH):
                        nc.scalar.activation(
                            kT[g][:, 512 * th:512 * (th + 1)], pk[(g, th)][:],
                            mybir.ActivationFunctionType.Identity,
                            bias=bias_sb[:, BOFF_K + g:BOFF_K + g + 1],
                            scale=1.0,
                        )
                def v_half(c, tb, pv2):
                    for i in range(2):
                        t = tb + i
                        nc.tensor.matmul(
                            pv2[i][:, 0:KF],
                            xsl(c, 128 * t, 128 * (t + 1)),
                            wv_sb[:, KF * c:KF * (c + 1)],
                            start=(c == 0), stop=(c == KC - 1),
                        )

                def v_half_out(tb, pv2):
                    for i in range(2):
                        t = tb + i
                        pv_j = pv2[i][:, 0:KF].rearrange(
                            "p (j f) -> p j f", j=NKV)
                        nc.vector.tensor_add(va[:, t, :, 0:64], pv_j, bv_j)

                # V tiles 4-7 in two 2-tile passes: the first reuses only
                # banks pva0/pva1, so it waits just the first two V03 adds.
                pv45 = [psV.tile([128, 512], F32, tag=f"pva{i}", name=f"pvb{i}")
                        for i in range(2)]
                for c in range(KC):
                    v_half(c, 4, pv45)
                v_half_out(4, pv45)
                pv67 = [psV.tile([128, 512], F32, tag=f"pva{i + 2}",
                                 name=f"pvc{i}")
                        for i in range(2)]
                for c in range(KC):
                    v_half(c, 6, pv67)
                v_half_out(6, pv67)

            # ---- Phase B (+ A2 Q pairs 0/1 + phase C t4-7, sharing pools)
            with (
                tc.tile_pool(name="epool", bufs=16) as ep,
                tc.tile_pool(name="npool", bufs=2) as npool,
                tc.tile_pool(name="pvsp", bufs=4) as pvsp,
                tc.tile_pool(name="osb", bufs=2) as osbp,
                tc.tile_pool(name="ps_sc", bufs=2, space="PSUM") as ps_sc,
                tc.tile_pool(name="ps_tr", bufs=2, space="PSUM") as ps_tr,
                tc.tile_pool(name="ps_fq", bufs=1, space="PSUM") as ps_fq,
                tc.tile_pool(name="ps_fo", bufs=1, space="PSUM") as ps_fo,
            ):
                fill_alt = [0]

                def fill_bank(name):
                    # Alternate the two 1-bank filler psums so consecutive
                    # accumulation groups double-buffer instead of
                    # serializing on the previous group's psum readout.
                    pool, tg = [(ps_fq, "fq"), (ps_fo, "fo")][fill_alt[0] % 2]
                    fill_alt[0] += 1
                    return pool.tile([128, 512], F32, tag=tg, name=name)

                def q_half_item(p, thh, dc, box):
                    """One DoubleRow d-chunk of the Q projection of
                    (pair p, half thh); bias-add to qT on the last chunk."""
                    if dc == 0:
                        box["pq"] = fill_bank(f"fq{p}_{thh}")
                    pq = box["pq"]
                    nc.tensor.matmul(
                        pq[:],
                        two(wq8_sb[:, 2048 * p + 256 * dc:
                                   2048 * p + 256 * (dc + 1)]),
                        x8dc(dc)[:, :, 512 * thh:512 * (thh + 1)],
                        start=(dc == 0), stop=(dc == DC - 1),
                        perf_mode=DRM,
                    )
                    if dc == DC - 1:
                        nc.vector.tensor_scalar_add(
                            qT[p][:, 512 * thh:512 * (thh + 1)], pq[:],
                            bias_sb[:, BOFF_Q + p:BOFF_Q + p + 1],
                        )

                o_tiles = {}

                def o_item(t, nf, k, box):
                    """One pair's contribution to out tile t, nf slot nf;
                    copy to o_sb on the last pair, store on the last nf.
                    Alternates the fo/fq psum banks so consecutive singles
                    double-buffer instead of serializing on the copy-out."""
                    if k == 0:
                        box["po"] = fill_bank(f"fo{t}_{nf}")
                    po = box["po"]
                    nc.tensor.matmul(
                        po[:],
                        ctxT[k][:, 128 * t:128 * (t + 1)],
                        wo_sb[:, 2048 * k + 512 * nf:2048 * k + 512 * (nf + 1)],
                        start=(k == 0), stop=(k == 7),
                    )
                    if k == 7:
                        if nf == 0:
                            o_tiles[t] = osbp.tile([128, D], F32, tag="osb",
                                                   name=f"osb{t}")
                        nc.vector.tensor_copy(
                            o_tiles[t][:, 512 * nf:512 * (nf + 1)], po[:])
                        if nf == 3:
                            nc.sync.dma_start(
                                out=out[128 * t:128 * (t + 1), :],
                                in_=o_tiles[t][:])

                def make_queue_filler(queue, slot_total):
                    # 18 calls per unit (2 in the tail between the last PVs);
                    # front-load zeros so the tail always has filler to hide
                    # the last exps' latency.
                    base = slot_total // 18
                    extra = slot_total - 18 * base
                    counts = [base + (1 if i >= 18 - extra else 0)
                              for i in range(18)]

                    def fill(i):
                        for _ in range(counts[i]):
                            if queue:
                                queue.pop(0)()
                    return fill

                # A2: Q pairs 0 and 1, th=0 halves, in one psc-tag tile
                # (keeps ps_fq free of WARs for the th0 sweep's first filler).
                pqA = ps_sc.tile([128, 1024], F32, tag="psc", name="pqA")
                for p in range(2):
                    for dc in range(DC):
                        nc.tensor.matmul(
                            pqA[:, 512 * p:512 * (p + 1)],
                            two(wq8_sb[:, 2048 * p + 256 * dc:
                                       2048 * p + 256 * (dc + 1)]),
                            x8dc(dc)[:, :, 0:512],
                            start=(dc == 0), stop=(dc == DC - 1),
                            perf_mode=DRM,
                        )
                    nc.scalar.activation(
                        qT[p][:, 0:512], pqA[:, 512 * p:512 * (p + 1)],
                        mybir.ActivationFunctionType.Identity,
                        bias=bias_sb[:, BOFF_Q + p:BOFF_Q + p + 1],
                        scale=1.0,
                    )

                def pv_pair(p, th, sqc, es, kvlo, kvhi):
                    """Query chunk sqc, both head halves: each accumulates
                    e.T @ vaug over the 8 key blocks into a [128, 65] psum
                    (col 64 = denominator) and normalizes into one half of a
                    shared [128, 128] staging tile; a single PE transpose
                    then restores both halves to feature-major ctxT."""
                    ctx2 = npool.tile([128, 128], BF16, tag="cs",
                                      name=f"cs{p}{th}{sqc}")
                    for hh, kv in ((0, kvlo), (1, kvhi)):
                        pvf = fill_bank(f"pv{p}_{th}_{sqc}_{hh}")
                        for blk in range(TT):
                            nc.tensor.matmul(
                                pvf[:, 0:65],
                                es[blk][:, 512 * hh + 128 * sqc:
                                        512 * hh + 128 * (sqc + 1)],
                                vaug[:, 260 * blk + 65 * kv:
                                     260 * blk + 65 * kv + 65],
                                start=(blk == 0), stop=(blk == TT - 1),
                            )
                        # one fast copy frees the psum bank; the slow
                        # recip+mul readout runs off SBUF staging instead of
                        # holding up the 2-deep bank rotation.
                        pvs = pvsp.tile([128, 65], F32, tag="pvs",
                                        name=f"pvs{p}{th}{sqc}{hh}")
                        nc.vector.tensor_copy(pvs[:], pvf[:, 0:65])
                        rec = npool.tile([128, 1], F32, tag="rec",
                                         name=f"r{p}{th}{sqc}{hh}")
                        nc.vector.reciprocal(rec[:], pvs[:, 64:65])
                        nc.vector.tensor_scalar_mul(
                            ctx2[:, 64 * hh:64 * (hh + 1)], pvs[:, 0:64], rec[:])
                    ptr = ps_tr.tile([128, 128], BF16, tag="ptr",
                                     name=f"ptr{p}{th}{sqc}")
                    nc.tensor.transpose(ptr[:], ctx2[:], tid[:])
                    nc.vector.tensor_copy(
                        ctxT[p][:, 512 * th + 128 * sqc:
                                512 * th + 128 * (sqc + 1)],
                        ptr[:])

                def unit(p, th, filler):
                    """Attention for (pair p, query half th), PV flipped.
                    `filler(i)`, i in 0..17, emits interleaved PE filler."""
                    glo, ghi = LO[p] // 4 // 2, HI[p] // 4 // 2
                    kvlo, kvhi = LO[p] // 4, HI[p] // 4
                    es = [None] * TT

                    for blk in range(TT):
                        psc = ps_sc.tile([128, 1024], F32, tag="psc",
                                         name=f"psc{p}_{th}_{blk}")
                        nc.tensor.matmul(
                            psc[:, 0:512],
                            kT[glo][0:64, 128 * blk:128 * (blk + 1)],
                            qT[p][0:64, 512 * th:512 * (th + 1)],
                            start=True, stop=True,
                        )
                        nc.tensor.matmul(
                            psc[:, 512:1024],
                            kT[ghi][64:128, 128 * blk:128 * (blk + 1)],
                            qT[p][64:128, 512 * th:512 * (th + 1)],
                            start=True, stop=True,
                        )
                        e = ep.tile([128, 1024], BF16, tag="e",
                                    name=f"e{p}_{th}_{blk}")
                        nc.scalar.activation(
                            e[:], psc[:], mybir.ActivationFunctionType.Exp,
                            bias=0.0, scale=SCALE8,
                        )
                        es[blk] = e
                        filler(2 * blk)
                        filler(2 * blk + 1)
                    # all PV groups read every e tile, so they start after
                    # exp(7); the two tail filler slots bridge that latency.
                    filler(16)
                    filler(17)
                    for sqc in range(4):
                        pv_pair(p, th, sqc, es, kvlo, kvhi)

                # --- th=0 sweep. Filler queue: Q th0-halves of pairs 2-7,
                # then Q th1-halves of pairs 0-1 (needed before th1 sweep).
                q0 = []
                for fp in range(2, 8):
                    box = {}
                    for dc in range(DC):
                        q0.append(lambda fp=fp, dc=dc, box=box:
                                  q_half_item(fp, 0, dc, box))
                for fp in list(range(2)) + list(range(2, 8)):
                    box = {}
                    for dc in range(DC):
                        q0.append(lambda fp=fp, dc=dc, box=box:
                                  q_half_item(fp, 1, dc, box))
                th0_counts = [20, 20, 14, 14, 14, 14, 11, 5]
                for p in range(8):
                    unit(p, 0, make_queue_filler(q0, th0_counts[p]))
                assert not q0

                # --- th=1 sweep. Filler queue: O projection of tiles 0-3.
                q1 = []
                for t in range(4):
                    for nf in range(4):
                        box = {}
                        for k in range(8):
                            q1.append(lambda t=t, nf=nf, k=k, box=box:
                                      o_item(t, nf, k, box))
                for p in [7] + list(range(7)):
                    unit(p, 1, make_queue_filler(q1, 16))
                assert not q1

                # --- Phase C: O proj of token tiles 4-7. The last tile
                # streams per-512-column: matmuls -> copy -> store per nf, so
                # the final store drains right behind the final matmul.
                for t in range(4, TT):
                    for half in range(2):
                        pt = ps_sc.tile([128, 1024], F32, tag="psc",
                                        name=f"poC{t}_{half}")
                        if half == 0:
                            o_tiles[t] = osbp.tile([128, D], F32, tag="osb",
                                                   name=f"osbC{t}")
                        for j in range(2):
                            nf = 2 * half + j
                            for k in range(8):
                                nc.tensor.matmul(
                                    pt[:, 512 * j:512 * (j + 1)],
                                    ctxT[k][:, 128 * t:128 * (t + 1)],
                                    wo_sb[:, 2048 * k + 512 * nf:
                                          2048 * k + 512 * (nf + 1)],
                                    start=(k == 0), stop=(k == 7),
                                )
                            sl = o_tiles[t][:, 512 * nf:512 * (nf + 1)]
                            if nf % 2 == 0:
                                nc.scalar.copy(sl, pt[:, 512 * j:512 * (j + 1)])
                            else:
                                nc.vector.tensor_copy(
                                    sl, pt[:, 512 * j:512 * (j + 1)])
                            # per-512-column stores: each slice drains right
                            # behind its copy, so the final store isn't
                            # queued behind a whole-tile transfer.
                            nc.sync.dma_start(
                                out=out[128 * t:128 * (t + 1),
                                        512 * nf:512 * (nf + 1)],
                                in_=sl)

    nc.compile()
    _CACHE["nc"] = nc
    return nc


def _prep_core_inputs(c, x, Wq, bq, Wk, bk, Wv, bv, Wo, bo):
    tp = c % 2
    b = c // 2
    hperm = [16 * tp + h for h in HEAD_PERM]

    # xT chunks: region[dp, 1024*c + t] = x[b][t, 128*c + dp]
    xr = np.ascontiguousarray(x[b].T).reshape(KC, 128, S).transpose(1, 0, 2)
    xr = xr.reshape(128, KC * S)

    wk_r = Wk[:, KF * tp:KF * (tp + 1)].reshape(KC, 128, KF)
    wk_r = wk_r.transpose(1, 0, 2).reshape(128, KC * KF)
    wv_r = Wv[:, KF * tp:KF * (tp + 1)].reshape(KC, 128, KF)
    wv_r = wv_r.transpose(1, 0, 2).reshape(128, KC * KF)

    f8 = mybir.dt.np(FP8)
    xT = np.ascontiguousarray(x[b].T)
    # x8[k, 2048*dc + 1024*i + n] = xT[256*dc + 128*i + k, n]
    x8 = xT.reshape(DC, 2, 128, S).transpose(2, 0, 1, 3).reshape(128, DC * 2048)
    wq_perm = WSCALE * Wq.reshape(D, 32, HD)[:, hperm, :].reshape(D, QF)
    # wq8[k, 2048*pr + 256*dc + 128*i + m] = 64*Wq_perm[256dc+128i+k, 128pr+m]
    wq8 = wq_perm.reshape(DC, 2, 128, 8, 128).transpose(2, 3, 0, 1, 4)
    wq8 = wq8.reshape(128, 8 * 2048)
    blob8 = np.concatenate(
        [x8[:, 0:4 * 2048], wq8[:, 0:2 * 2048],
         x8[:, 4 * 2048:8 * 2048], wq8[:, 2 * 2048:8 * 2048]],
        axis=1).astype(f8)

    wo_perm = Wo.reshape(32, HD, D)[hperm].reshape(QF, D)
    wo_r = wo_perm.reshape(8, 128, D).transpose(1, 0, 2).reshape(128, 8 * D)

    blob = np.concatenate(
        [wk_r[:, 0:KF],              # wkA
         xr[:, 0:S],                 # xA
         wk_r[:, KF:4 * KF],         # wkA1
         xr[:, S:2 * S],             # xB1
         xr[:, 2 * S:4 * S],         # xB2
         wk_r[:, 4 * KF:10 * KF],    # wkB
         xr[:, 4 * S:6 * S],         # xC1
         xr[:, 6 * S:8 * S],         # xC2
         wv_r,                       # wv
         wk_r[:, 10 * KF:16 * KF],   # wkC
         xr[:, 8 * S:12 * S],        # xD
         xr[:, 12 * S:16 * S],       # xE
         wo_r,
         np.eye(128, dtype=np.float32)],
        axis=1).astype(ml_dtypes.bfloat16)

    bq_c = WSCALE * bq.reshape(32, HD)[hperm].reshape(8, 128).T   # [128, 8]
    bk_c = bk[KF * tp:KF * (tp + 1)].reshape(2, 128).T            # [128, 2]
    bv_c = np.tile(bv[KF * tp:KF * (tp + 1)][None, :], (128, 1))  # [128, 256]
    ones = np.ones((128, 4), np.float32)
    biasd = np.concatenate([bq_c, bk_c, bv_c, ones], axis=1).astype(np.float32)

    return {
        "blob": np.ascontiguousarray(blob),
        "blob8": np.ascontiguousarray(blob8),
        "biasd": np.ascontiguousarray(biasd),
    }


def kernel(x, Wq, bq, Wk, bk, Wv, bv, Wo, bo):
    global LAST_RUN_NS
    nc = _build()
    in_maps = [
        _prep_core_inputs(c, x, Wq, bq, Wk, bk, Wv, bv, Wo, bo) for c in range(8)
    ]
    t0 = time.perf_counter_ns()
    res = run_bass_kernel_spmd(nc, in_maps, list(range(8)))
    LAST_RUN_NS = time.perf_counter_ns() - t0
    parts = [res.results[c]["out"] for c in range(8)]
    out = np.empty((4, S, D), np.float32)
    for b in range(4):
        out[b] = parts[2 * b] + parts[2 * b + 1] + bo[None, :]
    return out
